# revision 3
# baseline (speedup 1.0000x reference)
"""Bahdanau additive attention (sparse_attention nn_AttentionLayer) on 8 trn2 NeuronCores.

Math: for each batch b (one per core):
    Ws = enc @ W_a            [Te, He]
    Uh = dec @ U_a            [Td, He]
    scores[d, t] = sum_he V[he] * tanh(Ws[t, he] + Uh[d, he])
    e = softmax(scores, axis=t);  c = e @ enc
Returns (c, e) like the reference.

Strategy: tanh(a+b) is approximated by an odd Fourier sine series
    tanh(x) ~= sum_m C[m] sin(omega_m x),  omega_m = m*pi/L
so sin(omega(a+b)) = sinA*cosB + cosA*sinB separates per mode into TensorE
matmuls contracting He. This moves the O(Td*Te*He) nonlinearity bulk off the
ScalarE (which only evaluates per-side sin/cos: O(M*(Te+Td)*He)) onto the PE.
Range reduction for the Sin LUT ([-pi, pi] valid range) uses the fp32
magic-number rounding trick; cos(w*u) = sin(pi/2 - w*|u|) keeps args in range.

Fit residual + bf16 quantization give c rel err ~2e-3, e rel err ~5e-3.
"""
import numpy as np

import concourse.bass as bass
import concourse.bacc as bacc
import concourse.mybir as mybir
from concourse.tile import TileContext
from concourse.bass_utils import run_bass_kernel_spmd

B, TE, TD, HE = 8, 256, 256, 512
P = 128
KC = HE // P     # 4 he-chunks
TC = TE // P     # 2 t-chunks
DC = TD // P     # 2 d-chunks

FP32 = mybir.dt.float32
BF16 = mybir.dt.bfloat16
AF = mybir.ActivationFunctionType
ALU = mybir.AluOpType

MODES = 20
LWIN = 12.0
COEF = [1.249346159286752, -0.020857201699973488, 0.36061225355393123,
        -0.030396761857750965, 0.16644744108023768, -0.027901144927222506,
        0.08326716563222766, -0.019489686279277368, 0.04013832978671286,
        -0.009692641637033126, 0.01759392840771416, -0.002765736957276071,
        0.0038538052942665023, 0.0034738703682099777, -0.0008640410143163998,
        0.00288723030811317, -0.004890681943409681, 0.008935420234180574,
        -0.007012207130842028, 0.0028804646305746287]
MAGIC = float(np.float32(1.5 * 2 ** 23))
SHRINK = 1.0 - 1e-5


def build_nc():
    from concourse.masks import make_identity

    nc = bacc.Bacc("TRN2")
    enc_ext = nc.declare_dram_parameter("encoder_out_seq", [TE, HE], FP32, isOutput=False)
    dec_ext = nc.declare_dram_parameter("decoder_out_seq", [TD, HE], FP32, isOutput=False)
    W_ext = nc.declare_dram_parameter("W_a", [HE, HE], FP32, isOutput=False)
    U_ext = nc.declare_dram_parameter("U_a", [HE, HE], FP32, isOutput=False)
    V_ext = nc.declare_dram_parameter("V_a", [HE, 1], FP32, isOutput=False)
    c_ext = nc.declare_dram_parameter("out_c", [TD, HE], FP32, isOutput=True)
    e_ext = nc.declare_dram_parameter("out_e", [TD, TE], FP32, isOutput=True)

    with TileContext(nc) as tc, \
         tc.tile_pool(name="consts", bufs=1) as consts, \
         tc.tile_pool(name="persist", bufs=1) as persist, \
         tc.tile_pool(name="loadp", bufs=2) as loadp, \
         tc.tile_pool(name="work", bufs=2) as work, \
         tc.tile_pool(name="workb", bufs=3) as workb, \
         tc.tile_pool(name="psA", bufs=2, space="PSUM") as psA, \
         tc.tile_pool(name="psS", bufs=1, space="PSUM") as psS:

        ident = consts.tile([P, P], BF16, tag="ident")
        make_identity(nc, ident)
        halfpi = consts.tile([P, 1], FP32, tag="halfpi")
        nc.gpsimd.memset(halfpi, float(np.pi / 2))
        V_sb = consts.tile([P, KC], FP32, tag="vsb")
        for k in range(KC):
            nc.sync.dma_start(out=V_sb[:, k:k + 1], in_=V_ext[k * P:(k + 1) * P, :])

        # ---- load + cast inputs ----
        encb, decb = [], []
        for t in range(TC):
            ef = loadp.tile([P, HE], FP32, tag="encf")
            nc.sync.dma_start(out=ef, in_=enc_ext[t * P:(t + 1) * P, :])
            eb = persist.tile([P, HE], BF16, tag=f"encb{t}")
            nc.vector.tensor_copy(eb, ef)
            encb.append(eb)
        for d in range(DC):
            df = loadp.tile([P, HE], FP32, tag="decf")
            nc.sync.dma_start(out=df, in_=dec_ext[d * P:(d + 1) * P, :])
            db = persist.tile([P, HE], BF16, tag=f"decb{d}")
            nc.vector.tensor_copy(db, df)
            decb.append(db)
        Wb, Ub = [], []
        for k in range(KC):
            wf = loadp.tile([P, HE], FP32, tag="wf")
            nc.sync.dma_start(out=wf, in_=W_ext[k * P:(k + 1) * P, :])
            wb = persist.tile([P, HE], BF16, tag=f"wb{k}")
            nc.vector.tensor_copy(wb, wf)
            Wb.append(wb)
            uf = loadp.tile([P, HE], FP32, tag="uf")
            nc.sync.dma_start(out=uf, in_=U_ext[k * P:(k + 1) * P, :])
            ub = persist.tile([P, HE], BF16, tag=f"ub{k}")
            nc.vector.tensor_copy(ub, uf)
            Ub.append(ub)

        # ---- transposes: encT/decT [128 hin, KC*... free = k*TE + t] ----
        encT = persist.tile([P, KC * TE], BF16, tag="encT")
        decT = persist.tile([P, KC * TD], BF16, tag="decT")
        for k in range(KC):
            for t in range(TC):
                tp = psA.tile([P, P], BF16, tag="tp")
                nc.tensor.transpose(tp, encb[t][:, k * P:(k + 1) * P], ident)
                nc.vector.tensor_copy(encT[:, k * TE + t * P: k * TE + (t + 1) * P], tp)
            for d in range(DC):
                tp = psA.tile([P, P], BF16, tag="tp")
                nc.tensor.transpose(tp, decb[d][:, k * P:(k + 1) * P], ident)
                nc.vector.tensor_copy(decT[:, k * TD + d * P: k * TD + (d + 1) * P], tp)

        # ---- projections: WsT/UhT [128 he_out(chunk m), m*TE + t] f32 ----
        WsT = persist.tile([P, KC * TE], FP32, tag="WsT")
        UhT = persist.tile([P, KC * TD], FP32, tag="UhT")
        for m in range(KC):
            ps = psA.tile([P, TE], FP32, tag="proj")
            for k in range(KC):
                nc.tensor.matmul(ps, Wb[k][:, m * P:(m + 1) * P],
                                 encT[:, k * TE:(k + 1) * TE],
                                 start=(k == 0), stop=(k == KC - 1))
            nc.vector.tensor_copy(WsT[:, m * TE:(m + 1) * TE], ps)
            ps = psA.tile([P, TD], FP32, tag="proj")
            for k in range(KC):
                nc.tensor.matmul(ps, Ub[k][:, m * P:(m + 1) * P],
                                 decT[:, k * TD:(k + 1) * TD],
                                 start=(k == 0), stop=(k == KC - 1))
            nc.vector.tensor_copy(UhT[:, m * TD:(m + 1) * TD], ps)

        # ---- mode loop ----
        score_ps = [psS.tile([P, TE], FP32, tag=f"s{d}", name=f"score{d}") for d in range(DC)]
        NF = KC * TE  # 1024 free elems per side tile

        for mi in range(MODES):
            m = mi + 1
            Pm = float(np.float32(2.0 * LWIN / m))
            INVP = float(np.float32(1.0 / np.float32(Pm)))
            om = float((np.pi / LWIN) * m * SHRINK)
            cmv = workb.tile([P, KC], FP32, tag="cmv")
            nc.vector.tensor_scalar(out=cmv, in0=V_sb, scalar1=float(COEF[mi]),
                                    scalar2=None, op0=ALU.mult)

            # modes whose args already fit in [-pi, pi] skip range reduction
            # (|x| < 6.0 => |omega_m * x| <= pi for m <= LWIN/6)
            no_reduce = (m * 6.0 <= LWIN)
            sides = {}
            for nm, src in (("A", WsT), ("B", UhT)):
                if no_reduce:
                    u = src
                else:
                    t1 = work.tile([P, NF], FP32, tag=f"rr1{nm}")
                    nc.vector.tensor_scalar(out=t1, in0=src, scalar1=INVP, scalar2=MAGIC,
                                            op0=ALU.mult, op1=ALU.add)
                    t2 = work.tile([P, NF], FP32, tag=f"rr2{nm}")
                    nc.vector.tensor_scalar(out=t2, in0=t1, scalar1=MAGIC, scalar2=Pm,
                                            op0=ALU.subtract, op1=ALU.mult)
                    u = work.tile([P, NF], FP32, tag=f"u{nm}")
                    nc.vector.tensor_tensor(out=u, in0=src, in1=t2, op=ALU.subtract)
                w = work.tile([P, NF], FP32, tag=f"w{nm}")
                nc.vector.tensor_scalar(out=w.bitcast(mybir.dt.int32),
                                        in0=u.bitcast(mybir.dt.int32),
                                        scalar1=0x7FFFFFFF, scalar2=None,
                                        op0=ALU.bitwise_and)
                sin_t = workb.tile([P, NF], BF16, tag=f"sin{nm}")
                nc.scalar.activation(sin_t, u, AF.Sin, bias=0.0, scale=om)
                cos_t = workb.tile([P, NF], BF16, tag=f"cos{nm}")
                nc.scalar.activation(cos_t, w, AF.Sin, bias=halfpi[:, :], scale=-om)
                sides[nm] = (sin_t, cos_t)

            sinA, cosA = sides["A"]
            sinB, cosB = sides["B"]
            SA = workb.tile([P, NF], BF16, tag="SA")
            CA = workb.tile([P, NF], BF16, tag="CA")
            for k in range(KC):
                ksl = slice(k * TE, (k + 1) * TE)
                nc.vector.tensor_scalar(out=SA[:, ksl], in0=sinA[:, ksl],
                                        scalar1=cmv[:, k:k + 1], scalar2=None, op0=ALU.mult)
                nc.vector.tensor_scalar(out=CA[:, ksl], in0=cosA[:, ksl],
                                        scalar1=cmv[:, k:k + 1], scalar2=None, op0=ALU.mult)

            for d in range(DC):
                for k in range(KC):
                    bsl = slice(k * TD + d * P, k * TD + (d + 1) * P)
                    ksl = slice(k * TE, (k + 1) * TE)
                    first = (mi == 0 and k == 0)
                    last = (mi == MODES - 1 and k == KC - 1)
                    nc.tensor.matmul(score_ps[d], cosB[:, bsl], SA[:, ksl],
                                     start=first, stop=False)
                    nc.tensor.matmul(score_ps[d], sinB[:, bsl], CA[:, ksl],
                                     start=False, stop=last)

        # ---- softmax (over t, free axis) + outputs ----
        exp_bf, recips = [], []
        for d in range(DC):
            eb = persist.tile([P, TE], BF16, tag=f"exp{d}")
            denom = persist.tile([P, 1], FP32, tag=f"den{d}")
            nc.scalar.activation(eb, score_ps[d], AF.Exp, bias=0.0, scale=1.0,
                                 accum_out=denom)
            r = persist.tile([P, 1], FP32, tag=f"rec{d}")
            nc.vector.reciprocal(r, denom)
            e_f = work.tile([P, TE], FP32, tag="ef")
            nc.vector.tensor_scalar(out=e_f, in0=eb, scalar1=r[:, :], scalar2=None,
                                    op0=ALU.mult)
            nc.sync.dma_start(out=e_ext[d * P:(d + 1) * P, :], in_=e_f)
            exp_bf.append(eb)
            recips.append(r)

        # transpose exp -> expT[t-part, d-free]
        expT = [persist.tile([P, TD], BF16, tag=f"expT{t}", name=f"expT{t}") for t in range(TC)]
        for d in range(DC):
            for t in range(TC):
                tp = psA.tile([P, P], BF16, tag="tp")
                nc.tensor.transpose(tp, exp_bf[d][:, t * P:(t + 1) * P], ident)
                nc.vector.tensor_copy(expT[t][:, d * P:(d + 1) * P], tp)

        for d in range(DC):
            ctx = psA.tile([P, HE], FP32, tag="ctx")
            for t in range(TC):
                nc.tensor.matmul(ctx, expT[t][:, d * P:(d + 1) * P], encb[t],
                                 start=(t == 0), stop=(t == TC - 1))
            c_f = work.tile([P, HE], FP32, tag="cf")
            nc.vector.tensor_scalar(out=c_f, in0=ctx, scalar1=recips[d][:, :],
                                    scalar2=None, op0=ALU.mult)
            nc.sync.dma_start(out=c_ext[d * P:(d + 1) * P, :], in_=c_f)

    return nc


def kernel(**inputs):
    enc = np.ascontiguousarray(np.asarray(inputs["encoder_out_seq"], dtype=np.float32))
    dec = np.ascontiguousarray(np.asarray(inputs["decoder_out_seq"], dtype=np.float32))
    W_a = np.ascontiguousarray(np.asarray(inputs["W_a"], dtype=np.float32))
    U_a = np.ascontiguousarray(np.asarray(inputs["U_a"], dtype=np.float32))
    V_a = np.ascontiguousarray(np.asarray(inputs["V_a"], dtype=np.float32))

    nc = build_nc()
    nc.finalize()
    in_maps = [
        {"encoder_out_seq": enc[b], "decoder_out_seq": dec[b],
         "W_a": W_a, "U_a": U_a, "V_a": V_a}
        for b in range(B)
    ]
    res = run_bass_kernel_spmd(nc, in_maps, list(range(B)))
    c = np.stack([res.results[b]["out_c"] for b in range(B)]).astype(np.float32)
    e = np.stack([res.results[b]["out_e"] for b in range(B)]).astype(np.float32)
    return c, e


# revision 4
# speedup vs baseline: 1.4399x; 1.4399x over previous
"""Bahdanau additive attention (sparse_attention nn_AttentionLayer) on 8 trn2 NeuronCores.

Math: for each batch b (one per core):
    Ws = enc @ W_a            [Te, He]
    Uh = dec @ U_a            [Td, He]
    scores[d, t] = sum_he V[he] * tanh(Ws[t, he] + Uh[d, he])
    e = softmax(scores, axis=t);  c = e @ enc
Returns (c, e) like the reference.

Strategy: tanh(a+b) is approximated by an odd Fourier sine series
    tanh(x) ~= sum_m C[m] sin(omega_m x),  omega_m = m*pi/L
so sin(omega(a+b)) = sinA*cosB + cosA*sinB separates per mode into TensorE
matmuls contracting He. This moves the O(Td*Te*He) nonlinearity bulk off the
ScalarE (which only evaluates per-side sin/cos: O(M*(Te+Td)*He)) onto the PE.
Range reduction for the Sin LUT ([-pi, pi] valid range) uses the fp32
magic-number rounding trick; cos(w*u) = sin(pi/2 - w*|u|) keeps args in range.

Fit residual + bf16 quantization give c rel err ~2e-3, e rel err ~5e-3.
"""
import numpy as np

import concourse.bass as bass
import concourse.bacc as bacc
import concourse.mybir as mybir
from concourse.tile import TileContext
from concourse.bass_utils import run_bass_kernel_spmd

B, TE, TD, HE = 8, 256, 256, 512
P = 128
KC = HE // P     # 4 he-chunks
TC = TE // P     # 2 t-chunks
DC = TD // P     # 2 d-chunks

FP32 = mybir.dt.float32
BF16 = mybir.dt.float16  # 16-bit compute dtype (fp16: 10 mantissa bits)
AF = mybir.ActivationFunctionType
ALU = mybir.AluOpType

MODES = 14
LWIN = 11.5
COEF = [1.2503627023252708, -0.02384816913271362, 0.35411668959209397,
        -0.03349929743792301, 0.16862845414044292, -0.03321541320437065,
        0.07189804576746558, -0.012364639423278614, 0.04805395615277799,
        -0.027680035618110256, 0.005494846067447737, 0.04442296055319397,
        -0.04457323244102494, 0.021539861075386413]
MAGIC = float(np.float32(1.5 * 2 ** 23))
SHRINK = 1.0 - 1e-5


def build_nc():
    from concourse.masks import make_identity

    nc = bacc.Bacc("TRN2")
    enc_ext = nc.declare_dram_parameter("encoder_out_seq", [TE, HE], FP32, isOutput=False)
    dec_ext = nc.declare_dram_parameter("decoder_out_seq", [TD, HE], FP32, isOutput=False)
    W_ext = nc.declare_dram_parameter("W_a", [HE, HE], FP32, isOutput=False)
    U_ext = nc.declare_dram_parameter("U_a", [HE, HE], FP32, isOutput=False)
    V_ext = nc.declare_dram_parameter("V_a", [HE, 1], FP32, isOutput=False)
    c_ext = nc.declare_dram_parameter("out_c", [TD, HE], FP32, isOutput=True)
    e_ext = nc.declare_dram_parameter("out_e", [TD, TE], FP32, isOutput=True)

    with TileContext(nc) as tc, \
         tc.tile_pool(name="consts", bufs=1) as consts, \
         tc.tile_pool(name="persist", bufs=1) as persist, \
         tc.tile_pool(name="loadp", bufs=2) as loadp, \
         tc.tile_pool(name="work", bufs=2) as work, \
         tc.tile_pool(name="workb", bufs=3) as workb, \
         tc.tile_pool(name="psA", bufs=2, space="PSUM") as psA, \
         tc.tile_pool(name="psS", bufs=1, space="PSUM") as psS:

        ident = consts.tile([P, P], BF16, tag="ident")
        make_identity(nc, ident)
        halfpi = consts.tile([P, 1], FP32, tag="halfpi")
        nc.gpsimd.memset(halfpi, float(np.pi / 2))
        V_sb = consts.tile([P, KC], FP32, tag="vsb")
        for k in range(KC):
            nc.sync.dma_start(out=V_sb[:, k:k + 1], in_=V_ext[k * P:(k + 1) * P, :])

        # ---- load + cast inputs ----
        encb, decb = [], []
        for t in range(TC):
            ef = loadp.tile([P, HE], FP32, tag="encf")
            nc.sync.dma_start(out=ef, in_=enc_ext[t * P:(t + 1) * P, :])
            eb = persist.tile([P, HE], BF16, tag=f"encb{t}")
            nc.vector.tensor_copy(eb, ef)
            encb.append(eb)
        for d in range(DC):
            df = loadp.tile([P, HE], FP32, tag="decf")
            nc.sync.dma_start(out=df, in_=dec_ext[d * P:(d + 1) * P, :])
            db = persist.tile([P, HE], BF16, tag=f"decb{d}")
            nc.vector.tensor_copy(db, df)
            decb.append(db)
        Wb, Ub = [], []
        for k in range(KC):
            wf = loadp.tile([P, HE], FP32, tag="wf")
            nc.sync.dma_start(out=wf, in_=W_ext[k * P:(k + 1) * P, :])
            wb = persist.tile([P, HE], BF16, tag=f"wb{k}")
            nc.vector.tensor_copy(wb, wf)
            Wb.append(wb)
            uf = loadp.tile([P, HE], FP32, tag="uf")
            nc.sync.dma_start(out=uf, in_=U_ext[k * P:(k + 1) * P, :])
            ub = persist.tile([P, HE], BF16, tag=f"ub{k}")
            nc.vector.tensor_copy(ub, uf)
            Ub.append(ub)

        # ---- transposes: encT/decT [128 hin, KC*... free = k*TE + t] ----
        encT = persist.tile([P, KC * TE], BF16, tag="encT")
        decT = persist.tile([P, KC * TD], BF16, tag="decT")
        for k in range(KC):
            for t in range(TC):
                tp = psA.tile([P, P], BF16, tag="tp")
                nc.tensor.transpose(tp, encb[t][:, k * P:(k + 1) * P], ident)
                nc.vector.tensor_copy(encT[:, k * TE + t * P: k * TE + (t + 1) * P], tp)
            for d in range(DC):
                tp = psA.tile([P, P], BF16, tag="tp")
                nc.tensor.transpose(tp, decb[d][:, k * P:(k + 1) * P], ident)
                nc.vector.tensor_copy(decT[:, k * TD + d * P: k * TD + (d + 1) * P], tp)

        # ---- projections: WsT/UhT [128 he_out(chunk m), m*TE + t] f32 ----
        WsT = persist.tile([P, KC * TE], FP32, tag="WsT")
        UhT = persist.tile([P, KC * TD], FP32, tag="UhT")
        for m in range(KC):
            ps = psA.tile([P, TE], FP32, tag="proj")
            for k in range(KC):
                nc.tensor.matmul(ps, Wb[k][:, m * P:(m + 1) * P],
                                 encT[:, k * TE:(k + 1) * TE],
                                 start=(k == 0), stop=(k == KC - 1))
            nc.vector.tensor_copy(WsT[:, m * TE:(m + 1) * TE], ps)
            ps = psA.tile([P, TD], FP32, tag="proj")
            for k in range(KC):
                nc.tensor.matmul(ps, Ub[k][:, m * P:(m + 1) * P],
                                 decT[:, k * TD:(k + 1) * TD],
                                 start=(k == 0), stop=(k == KC - 1))
            nc.vector.tensor_copy(UhT[:, m * TD:(m + 1) * TD], ps)

        # ---- mode loop ----
        score_ps = [psS.tile([P, TE], FP32, tag=f"s{d}", name=f"score{d}") for d in range(DC)]
        NF = KC * TE  # 1024 free elems per side tile

        for mi in range(MODES):
            m = mi + 1
            Pm = float(np.float32(2.0 * LWIN / m))
            INVP = float(np.float32(1.0 / np.float32(Pm)))
            om = float((np.pi / LWIN) * m * SHRINK)
            cmv = workb.tile([P, KC], FP32, tag="cmv")
            nc.vector.tensor_scalar(out=cmv, in0=V_sb, scalar1=float(COEF[mi]),
                                    scalar2=None, op0=ALU.mult)

            # modes whose args already fit in [-pi, pi] skip range reduction
            # (|x| < 6.0 => |omega_m * x| <= pi for m <= LWIN/6)
            no_reduce = (m * 6.0 <= LWIN)
            sides = {}
            for nm, src in (("A", WsT), ("B", UhT)):
                if no_reduce:
                    u = src
                else:
                    t1 = work.tile([P, NF], FP32, tag=f"rr1{nm}")
                    nc.vector.tensor_scalar(out=t1, in0=src, scalar1=INVP, scalar2=MAGIC,
                                            op0=ALU.mult, op1=ALU.add)
                    t2 = work.tile([P, NF], FP32, tag=f"rr2{nm}")
                    nc.vector.tensor_scalar(out=t2, in0=t1, scalar1=MAGIC, scalar2=Pm,
                                            op0=ALU.subtract, op1=ALU.mult)
                    u = work.tile([P, NF], FP32, tag=f"u{nm}")
                    nc.vector.tensor_tensor(out=u, in0=src, in1=t2, op=ALU.subtract)
                w = work.tile([P, NF], FP32, tag=f"w{nm}")
                nc.vector.tensor_scalar(out=w.bitcast(mybir.dt.int32),
                                        in0=u.bitcast(mybir.dt.int32),
                                        scalar1=0x7FFFFFFF, scalar2=None,
                                        op0=ALU.bitwise_and)
                sin_t = workb.tile([P, NF], BF16, tag=f"sin{nm}")
                nc.scalar.activation(sin_t, u, AF.Sin, bias=0.0, scale=om)
                cos_t = workb.tile([P, NF], BF16, tag=f"cos{nm}")
                nc.scalar.activation(cos_t, w, AF.Sin, bias=halfpi[:, :], scale=-om)
                sides[nm] = (sin_t, cos_t)

            sinA, cosA = sides["A"]
            sinB, cosB = sides["B"]
            SA = workb.tile([P, NF], BF16, tag="SA")
            CA = workb.tile([P, NF], BF16, tag="CA")
            for k in range(KC):
                ksl = slice(k * TE, (k + 1) * TE)
                nc.vector.tensor_scalar(out=SA[:, ksl], in0=sinA[:, ksl],
                                        scalar1=cmv[:, k:k + 1], scalar2=None, op0=ALU.mult)
                nc.vector.tensor_scalar(out=CA[:, ksl], in0=cosA[:, ksl],
                                        scalar1=cmv[:, k:k + 1], scalar2=None, op0=ALU.mult)

            for d in range(DC):
                for k in range(KC):
                    bsl = slice(k * TD + d * P, k * TD + (d + 1) * P)
                    ksl = slice(k * TE, (k + 1) * TE)
                    first = (mi == 0 and k == 0)
                    last = (mi == MODES - 1 and k == KC - 1)
                    nc.tensor.matmul(score_ps[d], cosB[:, bsl], SA[:, ksl],
                                     start=first, stop=False)
                    nc.tensor.matmul(score_ps[d], sinB[:, bsl], CA[:, ksl],
                                     start=False, stop=last)

        # ---- softmax (over t, free axis) + outputs ----
        exp_bf, recips = [], []
        for d in range(DC):
            eb = persist.tile([P, TE], BF16, tag=f"exp{d}")
            denom = persist.tile([P, 1], FP32, tag=f"den{d}")
            nc.scalar.activation(eb, score_ps[d], AF.Exp, bias=0.0, scale=1.0,
                                 accum_out=denom)
            r = persist.tile([P, 1], FP32, tag=f"rec{d}")
            nc.vector.reciprocal(r, denom)
            e_f = work.tile([P, TE], FP32, tag="ef")
            nc.vector.tensor_scalar(out=e_f, in0=eb, scalar1=r[:, :], scalar2=None,
                                    op0=ALU.mult)
            nc.sync.dma_start(out=e_ext[d * P:(d + 1) * P, :], in_=e_f)
            exp_bf.append(eb)
            recips.append(r)

        # transpose exp -> expT[t-part, d-free]
        expT = [persist.tile([P, TD], BF16, tag=f"expT{t}", name=f"expT{t}") for t in range(TC)]
        for d in range(DC):
            for t in range(TC):
                tp = psA.tile([P, P], BF16, tag="tp")
                nc.tensor.transpose(tp, exp_bf[d][:, t * P:(t + 1) * P], ident)
                nc.vector.tensor_copy(expT[t][:, d * P:(d + 1) * P], tp)

        for d in range(DC):
            ctx = psA.tile([P, HE], FP32, tag="ctx")
            for t in range(TC):
                nc.tensor.matmul(ctx, expT[t][:, d * P:(d + 1) * P], encb[t],
                                 start=(t == 0), stop=(t == TC - 1))
            c_f = work.tile([P, HE], FP32, tag="cf")
            nc.vector.tensor_scalar(out=c_f, in0=ctx, scalar1=recips[d][:, :],
                                    scalar2=None, op0=ALU.mult)
            nc.sync.dma_start(out=c_ext[d * P:(d + 1) * P, :], in_=c_f)

    return nc


def kernel(**inputs):
    enc = np.ascontiguousarray(np.asarray(inputs["encoder_out_seq"], dtype=np.float32))
    dec = np.ascontiguousarray(np.asarray(inputs["decoder_out_seq"], dtype=np.float32))
    W_a = np.ascontiguousarray(np.asarray(inputs["W_a"], dtype=np.float32))
    U_a = np.ascontiguousarray(np.asarray(inputs["U_a"], dtype=np.float32))
    V_a = np.ascontiguousarray(np.asarray(inputs["V_a"], dtype=np.float32))

    nc = build_nc()
    nc.finalize()
    in_maps = [
        {"encoder_out_seq": enc[b], "decoder_out_seq": dec[b],
         "W_a": W_a, "U_a": U_a, "V_a": V_a}
        for b in range(B)
    ]
    res = run_bass_kernel_spmd(nc, in_maps, list(range(B)))
    c = np.stack([res.results[b]["out_c"] for b in range(B)]).astype(np.float32)
    e = np.stack([res.results[b]["out_e"] for b in range(B)]).astype(np.float32)
    return c, e


# revision 5
# speedup vs baseline: 1.4445x; 1.0033x over previous
"""Bahdanau additive attention (sparse_attention nn_AttentionLayer) on 8 trn2 NeuronCores.

Math: for each batch b (one per core):
    Ws = enc @ W_a            [Te, He]
    Uh = dec @ U_a            [Td, He]
    scores[d, t] = sum_he V[he] * tanh(Ws[t, he] + Uh[d, he])
    e = softmax(scores, axis=t);  c = e @ enc
Returns (c, e) like the reference.

Strategy: tanh(a+b) is approximated by an odd Fourier sine series
    tanh(x) ~= sum_m C[m] sin(omega_m x),  omega_m = m*pi/L
so sin(omega(a+b)) = sinA*cosB + cosA*sinB separates per mode into TensorE
matmuls contracting He. This moves the O(Td*Te*He) nonlinearity bulk off the
ScalarE (which only evaluates per-side sin/cos: O(M*(Te+Td)*He)) onto the PE.
Range reduction for the Sin LUT uses the fp32 magic-number rounding trick.
Instead of sin/cos pairs we use the +-pi/4 phase pair s+- = sin(w x +- pi/4):
sin(w(a+b)) = s+(a)s+(b) - s-(a)s-(b). Args reach 1.25*pi where the HW LUT
still holds ~2.5e-3 (measured); saves the |u| pass entirely.

Fit residual + bf16 quantization give c rel err ~2e-3, e rel err ~5e-3.
"""
import numpy as np

import concourse.bass as bass
import concourse.bacc as bacc
import concourse.mybir as mybir
from concourse.tile import TileContext
from concourse.bass_utils import run_bass_kernel_spmd

B, TE, TD, HE = 8, 256, 256, 512
P = 128
KC = HE // P     # 4 he-chunks
TC = TE // P     # 2 t-chunks
DC = TD // P     # 2 d-chunks

FP32 = mybir.dt.float32
BF16 = mybir.dt.float16  # 16-bit compute dtype (fp16: 10 mantissa bits)
AF = mybir.ActivationFunctionType
ALU = mybir.AluOpType

MODES = 14
LWIN = 11.5
COEF = [1.2503627023252708, -0.02384816913271362, 0.35411668959209397,
        -0.03349929743792301, 0.16862845414044292, -0.03321541320437065,
        0.07189804576746558, -0.012364639423278614, 0.04805395615277799,
        -0.027680035618110256, 0.005494846067447737, 0.04442296055319397,
        -0.04457323244102494, 0.021539861075386413]
MAGIC = float(np.float32(1.5 * 2 ** 23))
SHRINK = 1.0 - 1e-5


def build_nc():
    from concourse.masks import make_identity

    nc = bacc.Bacc("TRN2")
    enc_ext = nc.declare_dram_parameter("encoder_out_seq", [TE, HE], FP32, isOutput=False)
    dec_ext = nc.declare_dram_parameter("decoder_out_seq", [TD, HE], FP32, isOutput=False)
    W_ext = nc.declare_dram_parameter("W_a", [HE, HE], FP32, isOutput=False)
    U_ext = nc.declare_dram_parameter("U_a", [HE, HE], FP32, isOutput=False)
    V_ext = nc.declare_dram_parameter("V_a", [HE, 1], FP32, isOutput=False)
    c_ext = nc.declare_dram_parameter("out_c", [TD, HE], FP32, isOutput=True)
    e_ext = nc.declare_dram_parameter("out_e", [TD, TE], FP32, isOutput=True)

    with TileContext(nc) as tc, \
         tc.tile_pool(name="consts", bufs=1) as consts, \
         tc.tile_pool(name="persist", bufs=1) as persist, \
         tc.tile_pool(name="loadp", bufs=2) as loadp, \
         tc.tile_pool(name="work", bufs=2) as work, \
         tc.tile_pool(name="workb", bufs=3) as workb, \
         tc.tile_pool(name="psA", bufs=2, space="PSUM") as psA, \
         tc.tile_pool(name="psS", bufs=1, space="PSUM") as psS:

        ident = consts.tile([P, P], BF16, tag="ident")
        make_identity(nc, ident)
        qpi = consts.tile([P, 1], FP32, tag="qpi")
        nc.gpsimd.memset(qpi, float(np.pi / 4))
        mqpi = consts.tile([P, 1], FP32, tag="mqpi")
        nc.gpsimd.memset(mqpi, float(-np.pi / 4))
        V_sb = consts.tile([P, KC], FP32, tag="vsb")
        for k in range(KC):
            nc.sync.dma_start(out=V_sb[:, k:k + 1], in_=V_ext[k * P:(k + 1) * P, :])

        # ---- load + cast inputs ----
        encb, decb = [], []
        for t in range(TC):
            ef = loadp.tile([P, HE], FP32, tag="encf")
            nc.sync.dma_start(out=ef, in_=enc_ext[t * P:(t + 1) * P, :])
            eb = persist.tile([P, HE], BF16, tag=f"encb{t}")
            nc.vector.tensor_copy(eb, ef)
            encb.append(eb)
        for d in range(DC):
            df = loadp.tile([P, HE], FP32, tag="decf")
            nc.sync.dma_start(out=df, in_=dec_ext[d * P:(d + 1) * P, :])
            db = persist.tile([P, HE], BF16, tag=f"decb{d}")
            nc.vector.tensor_copy(db, df)
            decb.append(db)
        Wb, Ub = [], []
        for k in range(KC):
            wf = loadp.tile([P, HE], FP32, tag="wf")
            nc.sync.dma_start(out=wf, in_=W_ext[k * P:(k + 1) * P, :])
            wb = persist.tile([P, HE], BF16, tag=f"wb{k}")
            nc.vector.tensor_copy(wb, wf)
            Wb.append(wb)
            uf = loadp.tile([P, HE], FP32, tag="uf")
            nc.sync.dma_start(out=uf, in_=U_ext[k * P:(k + 1) * P, :])
            ub = persist.tile([P, HE], BF16, tag=f"ub{k}")
            nc.vector.tensor_copy(ub, uf)
            Ub.append(ub)

        # ---- transposes: encT/decT [128 hin, KC*... free = k*TE + t] ----
        encT = persist.tile([P, KC * TE], BF16, tag="encT")
        decT = persist.tile([P, KC * TD], BF16, tag="decT")
        for k in range(KC):
            for t in range(TC):
                tp = psA.tile([P, P], BF16, tag="tp")
                nc.tensor.transpose(tp, encb[t][:, k * P:(k + 1) * P], ident)
                nc.vector.tensor_copy(encT[:, k * TE + t * P: k * TE + (t + 1) * P], tp)
            for d in range(DC):
                tp = psA.tile([P, P], BF16, tag="tp")
                nc.tensor.transpose(tp, decb[d][:, k * P:(k + 1) * P], ident)
                nc.vector.tensor_copy(decT[:, k * TD + d * P: k * TD + (d + 1) * P], tp)

        # ---- projections: WsT/UhT [128 he_out(chunk m), m*TE + t] f32 ----
        WsT = persist.tile([P, KC * TE], FP32, tag="WsT")
        UhT = persist.tile([P, KC * TD], FP32, tag="UhT")
        for m in range(KC):
            ps = psA.tile([P, TE], FP32, tag="proj")
            for k in range(KC):
                nc.tensor.matmul(ps, Wb[k][:, m * P:(m + 1) * P],
                                 encT[:, k * TE:(k + 1) * TE],
                                 start=(k == 0), stop=(k == KC - 1))
            nc.vector.tensor_copy(WsT[:, m * TE:(m + 1) * TE], ps)
            ps = psA.tile([P, TD], FP32, tag="proj")
            for k in range(KC):
                nc.tensor.matmul(ps, Ub[k][:, m * P:(m + 1) * P],
                                 decT[:, k * TD:(k + 1) * TD],
                                 start=(k == 0), stop=(k == KC - 1))
            nc.vector.tensor_copy(UhT[:, m * TD:(m + 1) * TD], ps)

        # ---- mode loop ----
        score_ps = [psS.tile([P, TE], FP32, tag=f"s{d}", name=f"score{d}") for d in range(DC)]
        NF = KC * TE  # 1024 free elems per side tile

        for mi in range(MODES):
            m = mi + 1
            Pm = float(np.float32(2.0 * LWIN / m))
            INVP = float(np.float32(1.0 / np.float32(Pm)))
            om = float((np.pi / LWIN) * m * SHRINK)
            cmv = workb.tile([P, KC], FP32, tag="cmv")
            nc.vector.tensor_scalar(out=cmv, in0=V_sb, scalar1=float(COEF[mi]),
                                    scalar2=None, op0=ALU.mult)
            cmvn = workb.tile([P, KC], FP32, tag="cmvn")
            nc.vector.tensor_scalar(out=cmvn, in0=V_sb, scalar1=float(-COEF[mi]),
                                    scalar2=None, op0=ALU.mult)

            # modes whose args already fit in [-pi, pi] skip range reduction
            # (|x| < 6.0 => |omega_m * x| <= pi for m <= LWIN/6)
            no_reduce = (m * 6.0 <= LWIN)
            sides = {}
            for nm, src in (("A", WsT), ("B", UhT)):
                if no_reduce:
                    u = src
                else:
                    t1 = work.tile([P, NF], FP32, tag=f"rr1{nm}")
                    nc.vector.tensor_scalar(out=t1, in0=src, scalar1=INVP, scalar2=MAGIC,
                                            op0=ALU.mult, op1=ALU.add)
                    t2 = work.tile([P, NF], FP32, tag=f"rr2{nm}")
                    nc.vector.tensor_scalar(out=t2, in0=t1, scalar1=MAGIC, scalar2=Pm,
                                            op0=ALU.subtract, op1=ALU.mult)
                    u = work.tile([P, NF], FP32, tag=f"u{nm}")
                    nc.vector.tensor_tensor(out=u, in0=src, in1=t2, op=ALU.subtract)
                sp_t = workb.tile([P, NF], BF16, tag=f"sp{nm}")
                nc.scalar.activation(sp_t, u, AF.Sin, bias=qpi[:, :], scale=om)
                sm_t = workb.tile([P, NF], BF16, tag=f"sm{nm}")
                nc.scalar.activation(sm_t, u, AF.Sin, bias=mqpi[:, :], scale=om)
                sides[nm] = (sp_t, sm_t)

            spA, smA = sides["A"]
            spB, smB = sides["B"]
            SP = workb.tile([P, NF], BF16, tag="SP")
            SM = workb.tile([P, NF], BF16, tag="SM")
            for k in range(KC):
                ksl = slice(k * TE, (k + 1) * TE)
                nc.vector.tensor_scalar(out=SP[:, ksl], in0=spA[:, ksl],
                                        scalar1=cmv[:, k:k + 1], scalar2=None, op0=ALU.mult)
                nc.vector.tensor_scalar(out=SM[:, ksl], in0=smA[:, ksl],
                                        scalar1=cmvn[:, k:k + 1], scalar2=None, op0=ALU.mult)

            for d in range(DC):
                for k in range(KC):
                    bsl = slice(k * TD + d * P, k * TD + (d + 1) * P)
                    ksl = slice(k * TE, (k + 1) * TE)
                    first = (mi == 0 and k == 0)
                    last = (mi == MODES - 1 and k == KC - 1)
                    nc.tensor.matmul(score_ps[d], spB[:, bsl], SP[:, ksl],
                                     start=first, stop=False)
                    nc.tensor.matmul(score_ps[d], smB[:, bsl], SM[:, ksl],
                                     start=False, stop=last)

        # ---- softmax (over t, free axis) + outputs ----
        exp_bf, recips = [], []
        for d in range(DC):
            eb = persist.tile([P, TE], BF16, tag=f"exp{d}")
            denom = persist.tile([P, 1], FP32, tag=f"den{d}")
            nc.scalar.activation(eb, score_ps[d], AF.Exp, bias=0.0, scale=1.0,
                                 accum_out=denom)
            r = persist.tile([P, 1], FP32, tag=f"rec{d}")
            nc.vector.reciprocal(r, denom)
            e_f = work.tile([P, TE], FP32, tag="ef")
            nc.vector.tensor_scalar(out=e_f, in0=eb, scalar1=r[:, :], scalar2=None,
                                    op0=ALU.mult)
            nc.sync.dma_start(out=e_ext[d * P:(d + 1) * P, :], in_=e_f)
            exp_bf.append(eb)
            recips.append(r)

        # transpose exp -> expT[t-part, d-free]
        expT = [persist.tile([P, TD], BF16, tag=f"expT{t}", name=f"expT{t}") for t in range(TC)]
        for d in range(DC):
            for t in range(TC):
                tp = psA.tile([P, P], BF16, tag="tp")
                nc.tensor.transpose(tp, exp_bf[d][:, t * P:(t + 1) * P], ident)
                nc.vector.tensor_copy(expT[t][:, d * P:(d + 1) * P], tp)

        for d in range(DC):
            ctx = psA.tile([P, HE], FP32, tag="ctx")
            for t in range(TC):
                nc.tensor.matmul(ctx, expT[t][:, d * P:(d + 1) * P], encb[t],
                                 start=(t == 0), stop=(t == TC - 1))
            c_f = work.tile([P, HE], FP32, tag="cf")
            nc.vector.tensor_scalar(out=c_f, in0=ctx, scalar1=recips[d][:, :],
                                    scalar2=None, op0=ALU.mult)
            nc.sync.dma_start(out=c_ext[d * P:(d + 1) * P, :], in_=c_f)

    return nc


def kernel(**inputs):
    enc = np.ascontiguousarray(np.asarray(inputs["encoder_out_seq"], dtype=np.float32))
    dec = np.ascontiguousarray(np.asarray(inputs["decoder_out_seq"], dtype=np.float32))
    W_a = np.ascontiguousarray(np.asarray(inputs["W_a"], dtype=np.float32))
    U_a = np.ascontiguousarray(np.asarray(inputs["U_a"], dtype=np.float32))
    V_a = np.ascontiguousarray(np.asarray(inputs["V_a"], dtype=np.float32))

    nc = build_nc()
    nc.finalize()
    in_maps = [
        {"encoder_out_seq": enc[b], "decoder_out_seq": dec[b],
         "W_a": W_a, "U_a": U_a, "V_a": V_a}
        for b in range(B)
    ]
    res = run_bass_kernel_spmd(nc, in_maps, list(range(B)))
    c = np.stack([res.results[b]["out_c"] for b in range(B)]).astype(np.float32)
    e = np.stack([res.results[b]["out_e"] for b in range(B)]).astype(np.float32)
    return c, e


# revision 8
# speedup vs baseline: 1.8174x; 1.2581x over previous
"""Bahdanau additive attention (sparse_attention nn_AttentionLayer) on 8 trn2 NeuronCores.

Math: for each batch b (one per core):
    Ws = enc @ W_a            [Te, He]
    Uh = dec @ U_a            [Td, He]
    scores[d, t] = sum_he V[he] * tanh(Ws[t, he] + Uh[d, he])
    e = softmax(scores, axis=t);  c = e @ enc
Returns (c, e) like the reference.

Strategy: tanh(a+b) is approximated by an odd Fourier sine series
    tanh(x) ~= sum_m C[m] sin(omega_m x),  omega_m = m*pi/L
so sin(omega(a+b)) = sinA*cosB + cosA*sinB separates per mode into TensorE
matmuls contracting He. This moves the O(Td*Te*He) nonlinearity bulk off the
ScalarE (which only evaluates per-side sin/cos: O(M*(Te+Td)*He)) onto the PE.
Range reduction for the Sin LUT uses the fp32 magic-number rounding trick.
Instead of sin/cos pairs we use the +-pi/4 phase pair s+- = sin(w x +- pi/4):
sin(w(a+b)) = s+(a)s+(b) - s-(a)s-(b). Args reach 1.25*pi where the HW LUT
still holds ~2.5e-3 (measured); saves the |u| pass entirely.

Fit residual + bf16 quantization give c rel err ~2e-3, e rel err ~5e-3.
"""
import numpy as np

import concourse.bass as bass
import concourse.bacc as bacc
import concourse.mybir as mybir
from concourse.tile import TileContext
from concourse.bass_utils import run_bass_kernel_spmd

B, TE, TD, HE = 8, 256, 256, 512
P = 128
KC = HE // P     # 4 he-chunks
TC = TE // P     # 2 t-chunks
DC = TD // P     # 2 d-chunks

FP32 = mybir.dt.float32
BF16 = mybir.dt.float16  # 16-bit compute dtype (fp16: 10 mantissa bits)
AF = mybir.ActivationFunctionType
ALU = mybir.AluOpType

MODES = 14
LWIN = 11.5
COEF = [1.2503627023252708, -0.02384816913271362, 0.35411668959209397,
        -0.03349929743792301, 0.16862845414044292, -0.03321541320437065,
        0.07189804576746558, -0.012364639423278614, 0.04805395615277799,
        -0.027680035618110256, 0.005494846067447737, 0.04442296055319397,
        -0.04457323244102494, 0.021539861075386413]
MAGIC = float(np.float32(1.5 * 2 ** 23))
SHRINK = 1.0 - 1e-5


def build_nc():
    from concourse.masks import make_identity

    nc = bacc.Bacc("TRN2")
    enc_ext = nc.declare_dram_parameter("encoder_out_seq", [TE, HE], BF16, isOutput=False)
    dec_ext = nc.declare_dram_parameter("decoder_out_seq", [TD, HE], BF16, isOutput=False)
    W_ext = nc.declare_dram_parameter("W_a", [HE, HE], BF16, isOutput=False)
    U_ext = nc.declare_dram_parameter("U_a", [HE, HE], BF16, isOutput=False)
    V_ext = nc.declare_dram_parameter("V_a", [HE, 1], FP32, isOutput=False)
    c_ext = nc.declare_dram_parameter("out_c", [TD, HE], FP32, isOutput=True)
    e_ext = nc.declare_dram_parameter("out_e", [TD, TE], FP32, isOutput=True)

    with TileContext(nc) as tc, \
         tc.tile_pool(name="consts", bufs=1) as consts, \
         tc.tile_pool(name="persist", bufs=1) as persist, \
         tc.tile_pool(name="loadp", bufs=2) as loadp, \
         tc.tile_pool(name="work", bufs=2) as work, \
         tc.tile_pool(name="workb", bufs=3) as workb, \
         tc.tile_pool(name="psA", bufs=2, space="PSUM") as psA, \
         tc.tile_pool(name="psS", bufs=1, space="PSUM") as psS:

        ident = consts.tile([P, P], BF16, tag="ident")
        make_identity(nc, ident)
        qpi = consts.tile([P, 1], FP32, tag="qpi")
        nc.gpsimd.memset(qpi, float(np.pi / 4))
        mqpi = consts.tile([P, 1], FP32, tag="mqpi")
        nc.gpsimd.memset(mqpi, float(-np.pi / 4))
        V_sb = consts.tile([P, KC], FP32, tag="vsb")
        for k in range(KC):
            nc.sync.dma_start(out=V_sb[:, k:k + 1], in_=V_ext[k * P:(k + 1) * P, :])

        # ---- load inputs (fp16 in DRAM; transposed forms via DMA xbar) ----
        encb = []
        for t in range(TC):
            eb = persist.tile([P, HE], BF16, tag=f"encb{t}")
            nc.sync.dma_start(out=eb, in_=enc_ext[t * P:(t + 1) * P, :])
            encb.append(eb)
        Wb, Ub = [], []
        for k in range(KC):
            wb = persist.tile([P, HE], BF16, tag=f"wb{k}")
            nc.sync.dma_start(out=wb, in_=W_ext[k * P:(k + 1) * P, :])
            Wb.append(wb)
            ub = persist.tile([P, HE], BF16, tag=f"ub{k}")
            nc.sync.dma_start(out=ub, in_=U_ext[k * P:(k + 1) * P, :])
            Ub.append(ub)

        encT = persist.tile([P, KC * TE], BF16, tag="encT")
        decT = persist.tile([P, KC * TD], BF16, tag="decT")
        for k in range(KC):
            nc.sync.dma_start_transpose(
                encT[:, k * TE:(k + 1) * TE], enc_ext[:, k * P:(k + 1) * P])
            nc.sync.dma_start_transpose(
                decT[:, k * TD:(k + 1) * TD], dec_ext[:, k * P:(k + 1) * P])

        # ---- projections into one combined tile: A-half = WsT, B-half = UhT ----
        WsUhT = persist.tile([P, 2 * KC * TE], FP32, tag="WsUhT")
        BOFF = KC * TE
        for m in range(KC):
            ps = psA.tile([P, TE], FP32, tag="proj")
            for k in range(KC):
                nc.tensor.matmul(ps, Wb[k][:, m * P:(m + 1) * P],
                                 encT[:, k * TE:(k + 1) * TE],
                                 start=(k == 0), stop=(k == KC - 1))
            nc.vector.tensor_copy(WsUhT[:, m * TE:(m + 1) * TE], ps)
            ps = psA.tile([P, TD], FP32, tag="proj")
            for k in range(KC):
                nc.tensor.matmul(ps, Ub[k][:, m * P:(m + 1) * P],
                                 decT[:, k * TD:(k + 1) * TD],
                                 start=(k == 0), stop=(k == KC - 1))
            nc.vector.tensor_copy(WsUhT[:, BOFF + m * TD:BOFF + (m + 1) * TD], ps)

        # ---- mode loop ----
        score_ps = [psS.tile([P, TE], FP32, tag=f"s{d}", name=f"score{d}") for d in range(DC)]
        NF = 2 * KC * TE  # 2048 free elems: A-half then B-half

        for mi in range(modes):
            m = mi + 1
            Pm = float(np.float32(2.0 * LWIN / m))
            INVP = float(np.float32(1.0 / np.float32(Pm)))
            om = float((np.pi / LWIN) * m * SHRINK)
            cmv = workb.tile([P, KC], FP32, tag="cmv")
            nc.vector.tensor_scalar(out=cmv, in0=V_sb, scalar1=float(COEF[mi]),
                                    scalar2=None, op0=ALU.mult)
            cmvn = workb.tile([P, KC], FP32, tag="cmvn")
            nc.vector.tensor_scalar(out=cmvn, in0=V_sb, scalar1=float(-COEF[mi]),
                                    scalar2=None, op0=ALU.mult)

            # modes whose args already fit in [-pi, pi] skip range reduction
            # (|x| < 6.0 => |omega_m * x| <= pi for m <= LWIN/6)
            no_reduce = (m * 6.0 <= LWIN) or no_reduce_all
            if no_reduce:
                u = WsUhT
            else:
                t1 = work.tile([P, NF], FP32, tag="rr1")
                nc.vector.tensor_scalar(out=t1, in0=WsUhT, scalar1=INVP, scalar2=MAGIC,
                                        op0=ALU.mult, op1=ALU.add)
                t2 = work.tile([P, NF], FP32, tag="rr2")
                nc.vector.tensor_scalar(out=t2, in0=t1, scalar1=MAGIC, scalar2=Pm,
                                        op0=ALU.subtract, op1=ALU.mult)
                u = work.tile([P, NF], FP32, tag="u")
                nc.vector.tensor_tensor(out=u, in0=WsUhT, in1=t2, op=ALU.subtract)
            sp_t = workb.tile([P, NF], BF16, tag="sp")
            nc.scalar.activation(sp_t, u, AF.Sin, bias=qpi[:, :], scale=om)
            sm_t = workb.tile([P, NF], BF16, tag="sm")
            nc.scalar.activation(sm_t, u, AF.Sin, bias=mqpi[:, :], scale=om)

            SP = workb.tile([P, BOFF], BF16, tag="SP")
            SM = workb.tile([P, BOFF], BF16, tag="SM")
            for k in range(KC):
                ksl = slice(k * TE, (k + 1) * TE)
                nc.vector.tensor_scalar(out=SP[:, ksl], in0=sp_t[:, ksl],
                                        scalar1=cmv[:, k:k + 1], scalar2=None, op0=ALU.mult)
                nc.vector.tensor_scalar(out=SM[:, ksl], in0=sm_t[:, ksl],
                                        scalar1=cmvn[:, k:k + 1], scalar2=None, op0=ALU.mult)

            for d in range(DC):
                for k in range(KC):
                    bsl = slice(BOFF + k * TD + d * P, BOFF + k * TD + (d + 1) * P)
                    ksl = slice(k * TE, (k + 1) * TE)
                    first = (mi == 0 and k == 0)
                    last = (mi == modes - 1 and k == KC - 1)
                    nc.tensor.matmul(score_ps[d], sp_t[:, bsl], SP[:, ksl],
                                     start=first, stop=False)
                    nc.tensor.matmul(score_ps[d], sm_t[:, bsl], SM[:, ksl],
                                     start=False, stop=last)

        # ---- softmax (over t, free axis) + outputs ----
        exp_bf, recips = [], []
        for d in range(DC):
            eb = persist.tile([P, TE], BF16, tag=f"exp{d}")
            denom = persist.tile([P, 1], FP32, tag=f"den{d}")
            nc.scalar.activation(eb, score_ps[d], AF.Exp, bias=0.0, scale=1.0,
                                 accum_out=denom)
            r = persist.tile([P, 1], FP32, tag=f"rec{d}")
            nc.vector.reciprocal(r, denom)
            e_f = work.tile([P, TE], FP32, tag="ef")
            nc.vector.tensor_scalar(out=e_f, in0=eb, scalar1=r[:, :], scalar2=None,
                                    op0=ALU.mult)
            nc.sync.dma_start(out=e_ext[d * P:(d + 1) * P, :], in_=e_f)
            exp_bf.append(eb)
            recips.append(r)

        # transpose exp -> expT[t-part, d-free]
        expT = [persist.tile([P, TD], BF16, tag=f"expT{t}", name=f"expT{t}") for t in range(TC)]
        for d in range(DC):
            for t in range(TC):
                tp = psA.tile([P, P], BF16, tag="tp")
                nc.tensor.transpose(tp, exp_bf[d][:, t * P:(t + 1) * P], ident)
                nc.vector.tensor_copy(expT[t][:, d * P:(d + 1) * P], tp)

        for d in range(DC):
            ctx = psA.tile([P, HE], FP32, tag="ctx")
            for t in range(TC):
                nc.tensor.matmul(ctx, expT[t][:, d * P:(d + 1) * P], encb[t],
                                 start=(t == 0), stop=(t == TC - 1))
            c_f = work.tile([P, HE], FP32, tag="cf")
            nc.vector.tensor_scalar(out=c_f, in0=ctx, scalar1=recips[d][:, :],
                                    scalar2=None, op0=ALU.mult)
            nc.sync.dma_start(out=c_ext[d * P:(d + 1) * P, :], in_=c_f)

    return nc


def make_in_maps(inputs):
    enc = np.asarray(inputs["encoder_out_seq"], dtype=np.float32).astype(np.float16)
    dec = np.asarray(inputs["decoder_out_seq"], dtype=np.float32).astype(np.float16)
    W_a = np.asarray(inputs["W_a"], dtype=np.float32).astype(np.float16)
    U_a = np.asarray(inputs["U_a"], dtype=np.float32).astype(np.float16)
    V_a = np.ascontiguousarray(np.asarray(inputs["V_a"], dtype=np.float32))
    return [
        {"encoder_out_seq": np.ascontiguousarray(enc[b]),
         "decoder_out_seq": np.ascontiguousarray(dec[b]),
         "W_a": W_a, "U_a": U_a, "V_a": V_a}
        for b in range(B)
    ]


def kernel(**inputs):
    nc = build_nc()
    nc.finalize()
    in_maps = make_in_maps(inputs)
    res = run_bass_kernel_spmd(nc, in_maps, list(range(B)))
    c = np.stack([res.results[b]["out_c"] for b in range(B)]).astype(np.float32)
    e = np.stack([res.results[b]["out_e"] for b in range(B)]).astype(np.float32)
    return c, e
